# revision 1
# baseline (speedup 1.0000x reference)
"""CourierEncoder fused kernel for 8 Trainium2 NeuronCores.

Data-parallel over the batch: each core processes B/8 = 32768 rows.
Per 512-row tile (all matmuls bf16 -> fp32 PSUM):
  embeds:  K=1 outer-product matmuls (w (x) coord), cos folded as Sin(z+pi/2),
           ACT Sin / Lrelu with exact per-partition f32 biases
  layer 1: feature-major, 6 matmuls [128,128]@[128,512]
  layer 2: batch-major (lhsT = h1T slices), bias b2 via ones (x) b2 matmul,
           LeakyReLU on DVE via scalar_tensor_tensor (max(0.01*z, z))
"""

import math

import numpy as np
import ml_dtypes

import concourse.bass as bass
import concourse.tile as tile
import concourse.mybir as mybir
from concourse import bacc
from concourse.bass_utils import run_bass_kernel_spmd

B = 262144
NCORES = 8
R = B // NCORES          # rows per core
TILE = 512               # rows per tile
NT = R // TILE           # tiles per core
PED = 256
NED = 128
CED = 256
Q = PED // 4             # 64
ALPHA = 0.01

F32 = mybir.dt.float32
BF16 = mybir.dt.bfloat16
AF = mybir.ActivationFunctionType
ALU = mybir.AluOpType

_CACHE = {}


def _build():
    nc = bacc.Bacc()
    xy = nc.dram_tensor("xy", [R, 2], F32, kind="ExternalInput")
    t = nc.dram_tensor("t", [R, 1], F32, kind="ExternalInput")
    emb_w = nc.dram_tensor("emb_w", [3, 128], BF16, kind="ExternalInput")
    biases = nc.dram_tensor("biases", [128, 5], F32, kind="ExternalInput")
    w1p = nc.dram_tensor("w1p", [128, 3, 2, 128], BF16, kind="ExternalInput")
    w2p = nc.dram_tensor("w2p", [128, 2, 256], BF16, kind="ExternalInput")
    b2rep = nc.dram_tensor("b2rep", [2, 512], BF16, kind="ExternalInput")
    out = nc.dram_tensor("out", [R, 256], F32, kind="ExternalOutput")

    with tile.TileContext(nc) as tc:
        with (
            tc.tile_pool(name="const", bufs=1) as const,
            tc.tile_pool(name="io", bufs=4) as io,
            tc.tile_pool(name="acts", bufs=3) as acts,
            tc.tile_pool(name="outp", bufs=4) as outp,
            tc.tile_pool(name="ps_emb", bufs=1, space="PSUM") as ps_emb,
            tc.tile_pool(name="ps_l1", bufs=1, space="PSUM") as ps_l1,
            tc.tile_pool(name="ps_l2", bufs=1, space="PSUM") as ps_l2,
        ):
            emb_w_sb = const.tile([65, 128], BF16)
            bias_sb = const.tile([128, 5], F32)
            w1_sb = const.tile([128, 3, 2, 128], BF16)
            w2_sb = const.tile([128, 2, 256], BF16)
            b2_sb = const.tile([2, 512], BF16)
            ones_sb = const.tile([2, 128], BF16)
            for c in range(3):
                nc.sync.dma_start(out=emb_w_sb[32 * c:32 * c + 1, :],
                                  in_=emb_w[c:c + 1, :])
            nc.sync.dma_start(out=bias_sb, in_=biases[:, :])
            nc.sync.dma_start(out=w1_sb, in_=w1p[:, :, :, :])
            nc.sync.dma_start(out=w2_sb, in_=w2p[:, :, :])
            nc.sync.dma_start(out=b2_sb, in_=b2rep[:, :])
            nc.vector.memset(ones_sb, 1.0)

            for it in range(NT):
                base = it * TILE
                # -- load + cast coords -------------------------------------
                xyt_f = io.tile([65, TILE], F32)
                xyt_b = io.tile([65, TILE], BF16)
                nc.sync.dma_start(
                    out=xyt_f[0:1, :],
                    in_=xy[base:base + TILE, 0:1].rearrange("n c -> c n"),
                )
                nc.sync.dma_start(
                    out=xyt_f[32:33, :],
                    in_=xy[base:base + TILE, 1:2].rearrange("n c -> c n"),
                )
                nc.sync.dma_start(
                    out=xyt_f[64:65, :],
                    in_=t[base:base + TILE, :].rearrange("n c -> c n"),
                )
                nc.vector.tensor_copy(out=xyt_b, in_=xyt_f)

                # -- embeddings (outer products) ----------------------------
                emb_ps = ps_emb.tile([128, 3, TILE], F32)
                for c in range(3):
                    nc.tensor.matmul(
                        emb_ps[:, c, :],
                        emb_w_sb[32 * c:32 * c + 1, :],
                        xyt_b[32 * c:32 * c + 1, :],
                        start=True, stop=True,
                    )
                hT = acts.tile([128, 3, TILE], BF16)
                nc.scalar.activation(out=hT[:, 0, :], in_=emb_ps[:, 0, :],
                                     func=AF.Sin, bias=bias_sb[:, 0:1])
                nc.scalar.activation(out=hT[:, 1, :], in_=emb_ps[:, 1, :],
                                     func=AF.Sin, bias=bias_sb[:, 1:2])
                nc.scalar.activation(out=hT[:, 2, :], in_=emb_ps[:, 2, :],
                                     func=AF.Prelu, bias=bias_sb[:, 2:3],
                                     alpha=ALPHA)

                # -- layer 1 (feature-major) --------------------------------
                l1_ps = ps_l1.tile([128, 2, TILE], F32)
                for mc in range(2):
                    for kc in range(3):
                        nc.tensor.matmul(
                            l1_ps[:, mc, :],
                            w1_sb[:, kc, mc, :],
                            hT[:, kc, :],
                            start=(kc == 0), stop=(kc == 2),
                        )
                h1T = acts.tile([128, 2, TILE], BF16)
                for mc in range(2):
                    nc.scalar.activation(out=h1T[:, mc, :], in_=l1_ps[:, mc, :],
                                         func=AF.Prelu, bias=bias_sb[:, 3 + mc:4 + mc],
                                         alpha=ALPHA)

                # -- layer 2 (batch-major) + LeakyReLU + store --------------
                o_sb = outp.tile([128, 4, 256], F32)
                l2_ps = ps_l2.tile([128, 4, 256], F32, tag="l2")
                for half in range(2):
                    nc.tensor.matmul(
                        l2_ps[:, 2 * half:2 * half + 2, :],
                        ones_sb[:, :],
                        b2_sb[:, :],
                        start=True, stop=False,
                        skip_group_check=True,
                    )
                for r in range(4):
                    for kc in range(2):
                        nc.tensor.matmul(
                            l2_ps[:, r, :],
                            h1T[:, kc, r * 128:(r + 1) * 128],
                            w2_sb[:, kc, :],
                            start=False, stop=(kc == 1),
                            skip_group_check=True,
                        )
                tmp_sb = outp.tile([128, 4, 256], F32, tag="l2tmp")
                nc.vector.tensor_scalar_mul(out=tmp_sb, in0=l2_ps, scalar1=ALPHA)
                nc.vector.tensor_max(out=o_sb, in0=l2_ps, in1=tmp_sb)
                nc.sync.dma_start(
                    out=out[base:base + TILE, :].rearrange("(r p) m -> p r m", p=128),
                    in_=o_sb,
                )
    nc.finalize()
    return nc


def _prep_weights(inputs):
    f = {k: np.asarray(v, dtype=np.float32) for k, v in inputs.items()}
    bf = ml_dtypes.bfloat16

    emb_w = np.stack([
        np.concatenate([f["w_sx"].ravel(), f["w_cx"].ravel()]),
        np.concatenate([f["w_sy"].ravel(), f["w_cy"].ravel()]),
        f["w_t"].ravel(),
    ]).astype(bf)

    biases = np.zeros((128, 5), np.float32)
    biases[:, 0] = np.concatenate([f["b_sx"], f["b_cx"] + math.pi / 2])
    biases[:, 1] = np.concatenate([f["b_sy"], f["b_cy"] + math.pi / 2])
    biases[:, 2] = f["b_t"]
    biases[:, 3] = f["b1"][0:128]
    biases[:, 4] = f["b1"][128:256]

    w1p = f["w1"].reshape(3, 128, 2, 128).transpose(1, 0, 2, 3).astype(bf)
    w2p = f["w2"].reshape(2, 128, 256).transpose(1, 0, 2).astype(bf)

    b2 = f["b2"]
    b2_hi = b2.astype(bf).astype(np.float32)
    b2_lo = b2 - b2_hi
    b2rep = np.stack([
        np.concatenate([b2_hi, b2_hi]),
        np.concatenate([b2_lo, b2_lo]),
    ]).astype(bf)

    return {
        "emb_w": emb_w,
        "biases": biases,
        "w1p": np.ascontiguousarray(w1p),
        "w2p": np.ascontiguousarray(w2p),
        "b2rep": b2rep,
    }


def kernel(**inputs):
    if "nc" not in _CACHE:
        _CACHE["nc"] = _build()
    nc = _CACHE["nc"]

    w = _prep_weights(inputs)
    xy = np.ascontiguousarray(np.asarray(inputs["xy"], dtype=np.float32))
    t = np.ascontiguousarray(np.asarray(inputs["t"], dtype=np.float32))

    in_maps = []
    for c in range(NCORES):
        lo, hi = c * R, (c + 1) * R
        in_maps.append({
            "xy": xy[lo:hi], "t": t[lo:hi], **w,
        })

    res = run_bass_kernel_spmd(nc, in_maps, core_ids=list(range(NCORES)))
    _CACHE["last_res"] = res
    return np.concatenate([res.results[c]["out"] for c in range(NCORES)], axis=0)



# revision 6
# speedup vs baseline: 1.0799x; 1.0799x over previous
"""CourierEncoder fused kernel for 8 Trainium2 NeuronCores.

Data-parallel over the batch: each core processes B/8 = 32768 rows.
Per 512-row tile (matmuls bf16 -> fp32 PSUM), software-pipelined over a
3-stage skew (embeds for tile a=k, layer 1 for b=k-1, layer 2 for c=k-2):
  embeds:  x/y as K=3 outer products {w, b_hi, b_lo} (x) {coord, 1, 1} at
           row strips 0/32 (concurrent via tile_position auto-derive);
           cos folded as Sin(z+pi/2); biases folded into the matmul so both
           Sin activations fuse into ONE scalar-engine op (FD=1024).
           t-embed via host-broadcast tb[128,R] + DVE tensor_scalar
           (per-partition w_t/b_t) + stt LeakyReLU -- no PE, no PSUM.
  b2 bias: ones (x) b2hi/lo matmuls at row strip 96/64, concurrent with
           the x/y embed matmuls (distinct 32-row strips).
  layer 1: feature-major, 6 matmuls [128,128]@[128,512]; bias+LeakyReLU on
           DVE via tensor_scalar_add (fp32 per-partition b1) + SBUF stt
  layer 2: batch-major (lhsT = h1T slices); LeakyReLU on ACT via one
           Prelu op (FD=1024, single PSUM input)
"""

import math

import numpy as np
import ml_dtypes

import concourse.bass as bass
import concourse.tile as tile
import concourse.mybir as mybir
from concourse import bacc
from concourse.bass_utils import run_bass_kernel_spmd

B = 262144
NCORES = 8
R = B // NCORES          # rows per core
TILE = 512               # rows per tile
NT = R // TILE           # tiles per core
G = 4                    # tiles per input DMA group
PED = 256
NED = 128
CED = 256
Q = PED // 4             # 64
ALPHA = 0.01

F32 = mybir.dt.float32
BF16 = mybir.dt.bfloat16
AF = mybir.ActivationFunctionType
ALU = mybir.AluOpType

_CACHE = {}


def _build():
    nc = bacc.Bacc()
    coords = nc.dram_tensor("coords", [6, R], BF16, kind="ExternalInput")
    tb = nc.dram_tensor("tb", [128, R], BF16, kind="ExternalInput")
    embw = nc.dram_tensor("embw", [35, 128], BF16, kind="ExternalInput")
    svec = nc.dram_tensor("svec", [128, 4], F32, kind="ExternalInput")
    w1p = nc.dram_tensor("w1p", [128, 3, 2, 128], BF16, kind="ExternalInput")
    w2p = nc.dram_tensor("w2p", [128, 2, 256], BF16, kind="ExternalInput")
    b2pack = nc.dram_tensor("b2pack", [2, 640], BF16, kind="ExternalInput")
    out = nc.dram_tensor("out", [R, 256], F32, kind="ExternalOutput")

    with tile.TileContext(nc) as tc:
        with (
            tc.tile_pool(name="const", bufs=1) as const,
            tc.tile_pool(name="io", bufs=2) as io,
            tc.tile_pool(name="acts", bufs=3) as acts,
            tc.tile_pool(name="outp", bufs=4) as outp,
            tc.tile_pool(name="ps_emb", bufs=1, space="PSUM") as ps_emb,
            tc.tile_pool(name="ps_l1", bufs=1, space="PSUM") as ps_l1,
            tc.tile_pool(name="ps_l2", bufs=2, space="PSUM") as ps_l2,
        ):
            embw_sb = const.tile([35, 128], BF16)
            sv_sb = const.tile([128, 4], F32)   # wt, bt, b1c0, b1c1
            w1_sb = const.tile([128, 3, 2, 128], BF16)
            w2_sb = const.tile([128, 2, 256], BF16)
            b2_sb = const.tile([98, 640], BF16)
            nc.sync.dma_start(out=embw_sb, in_=embw[:, :])
            nc.sync.dma_start(out=sv_sb, in_=svec[:, :])
            nc.sync.dma_start(out=w1_sb, in_=w1p[:, :, :, :])
            nc.sync.dma_start(out=w2_sb, in_=w2p[:, :, :])
            nc.sync.dma_start(out=b2_sb[96:98, :], in_=b2pack[:, :])
            nc.sync.dma_start(out=b2_sb[64:66, :], in_=b2pack[:, :])

            xyin = [None] * (NT // G)
            tin = [None] * (NT // G)
            hxy = [None] * NT
            ht_ = [None] * NT
            h1T = [None] * NT
            l1ps = [None] * NT
            l2ps = [None] * NT

            for k in range(NT + 2):
                a = k          # stage A: embeds
                b = k - 1      # stage B: layer 1
                c = k - 2      # stage C: layer 2 + store

                if a < NT:
                    ga, ja = divmod(a, G)
                    if ja == 0:
                        lo, hi = ga * G * 512, (ga + 1) * G * 512
                        xyin[ga] = io.tile([35, G, 512], BF16, tag="xyin", name="xyin")
                        tin[ga] = io.tile([128, G, 512], BF16, tag="tin", name="tin")
                        for cc in range(2):
                            nc.sync.dma_start(
                                out=xyin[ga][32 * cc:32 * cc + 3, :, :],
                                in_=coords[3 * cc:3 * cc + 3, lo:hi].rearrange(
                                    "p (g n) -> p g n", n=512),
                            )
                        nc.sync.dma_start(
                            out=tin[ga],
                            in_=tb[:, lo:hi].rearrange("p (g n) -> p g n", n=512),
                        )

                    # strip matmuls: x-emb(0), y-emb(32), b2 bias(64, 96)
                    if c >= 0:
                        l2ps[c] = ps_l2.tile([128, 4, 256], F32, tag="l2", name="l2ps")
                        nc.tensor.matmul(
                            l2ps[c][:, 0:2, :],
                            b2_sb[96:98, 0:128], b2_sb[96:98, 128:640],
                            start=True, stop=False,
                            skip_group_check=True, tile_position=(96, 0),
                        )
                        nc.tensor.matmul(
                            l2ps[c][:, 2:4, :],
                            b2_sb[64:66, 0:128], b2_sb[64:66, 128:640],
                            start=True, stop=False,
                            skip_group_check=True, tile_position=(64, 0),
                        )
                    emb_ps = ps_emb.tile([128, 2, 512], F32)
                    for cc in range(2):
                        nc.tensor.matmul(
                            emb_ps[:, cc, :],
                            embw_sb[32 * cc:32 * cc + 3, :],
                            xyin[ga][32 * cc:32 * cc + 3, ja, :],
                            start=True, stop=True,
                        )
                    hxy[a] = acts.tile([128, 2, 512], BF16, tag="hxy", name="hxy")
                    nc.scalar.activation(out=hxy[a], in_=emb_ps, func=AF.Sin)
                    # t-embed fully on DVE from broadcast tb
                    zt = acts.tile([128, 512], BF16, tag="zt")
                    ht_[a] = acts.tile([128, 512], BF16, tag="ht", name="ht")
                    nc.vector.tensor_scalar(
                        out=zt, in0=tin[ga][:, ja, :],
                        scalar1=sv_sb[:, 0:1], scalar2=sv_sb[:, 1:2],
                        op0=ALU.mult, op1=ALU.add)
                    nc.vector.scalar_tensor_tensor(
                        out=ht_[a], in0=zt, scalar=ALPHA, in1=zt,
                        op0=ALU.mult, op1=ALU.max)
                elif c >= 0:
                    l2ps[c] = ps_l2.tile([128, 4, 256], F32, tag="l2", name="l2ps")
                    for h in range(2):
                        nc.tensor.matmul(
                            l2ps[c][:, 2 * h:2 * h + 2, :],
                            b2_sb[96:98, 0:128], b2_sb[96:98, 128:640],
                            start=True, stop=False,
                            skip_group_check=True, tile_position=(96, 0),
                        )

                # -- stage B: layer 1 (feature-major) -----------------------
                if 0 <= b < NT:
                    l1ps[b] = ps_l1.tile([128, 2, 512], F32, name="l1ps")
                    for mc in range(2):
                        for kc in range(2):
                            nc.tensor.matmul(
                                l1ps[b][:, mc, :],
                                w1_sb[:, kc, mc, :],
                                hxy[b][:, kc, :],
                                start=(kc == 0), stop=False,
                            )
                        nc.tensor.matmul(
                            l1ps[b][:, mc, :],
                            w1_sb[:, 2, mc, :],
                            ht_[b],
                            start=False, stop=True,
                        )
                    h1T[b] = acts.tile([128, 2, 512], BF16, tag="h1T", name="h1T")
                    for mc in range(2):
                        tmp = acts.tile([128, 512], BF16, tag=f"tmp{mc}")
                        nc.vector.tensor_scalar_add(
                            out=tmp, in0=l1ps[b][:, mc, :],
                            scalar1=sv_sb[:, 2 + mc:3 + mc])
                        nc.vector.scalar_tensor_tensor(
                            out=h1T[b][:, mc, :], in0=tmp, scalar=ALPHA,
                            in1=tmp, op0=ALU.mult, op1=ALU.max)

                # -- stage C: layer 2 (batch-major) + LeakyReLU + store -----
                if c >= 0:
                    for r in range(4):
                        for kc in range(2):
                            nc.tensor.matmul(
                                l2ps[c][:, r, :],
                                h1T[c][:, kc, r * 128:(r + 1) * 128],
                                w2_sb[:, kc, :],
                                start=False, stop=(kc == 1),
                                skip_group_check=True,
                            )
                    o_sb = outp.tile([128, 4, 256], F32)
                    nc.scalar.activation(out=o_sb, in_=l2ps[c],
                                         func=AF.Prelu, alpha=ALPHA)
                    base = c * TILE
                    nc.sync.dma_start(
                        out=out[base:base + TILE, :].rearrange(
                            "(r p) m -> p r m", p=128),
                        in_=o_sb,
                    )
                    hxy[c] = ht_[c] = h1T[c] = l1ps[c] = l2ps[c] = None
    nc.finalize()
    return nc


def _prep_weights(inputs):
    f = {k: np.asarray(v, dtype=np.float32) for k, v in inputs.items()}
    bf = ml_dtypes.bfloat16

    def hilo(v):
        hi = v.astype(bf).astype(np.float32)
        return hi.astype(bf), (v - hi).astype(bf)

    embw = np.zeros((35, 128), bf)
    embw[0] = np.concatenate([f["w_sx"].ravel(), f["w_cx"].ravel()])
    bx = np.concatenate([f["b_sx"], f["b_cx"] + math.pi / 2])
    embw[1], embw[2] = hilo(bx)
    embw[32] = np.concatenate([f["w_sy"].ravel(), f["w_cy"].ravel()])
    by = np.concatenate([f["b_sy"], f["b_cy"] + math.pi / 2])
    embw[33], embw[34] = hilo(by)

    svec = np.stack([
        np.repeat(f["w_t"].ravel(), 1),
        f["b_t"],
        f["b1"][0:128],
        f["b1"][128:256],
    ], axis=1)
    svec = np.ascontiguousarray(svec, dtype=np.float32)

    w1p = f["w1"].reshape(3, 128, 2, 128).transpose(1, 0, 2, 3).astype(bf)
    w2p = f["w2"].reshape(2, 128, 256).transpose(1, 0, 2).astype(bf)

    b2hi, b2lo = hilo(f["b2"])
    b2pack = np.zeros((2, 640), bf)
    b2pack[:, 0:128] = 1.0
    b2pack[0, 128:640] = np.concatenate([b2hi, b2hi])
    b2pack[1, 128:640] = np.concatenate([b2lo, b2lo])

    return {
        "embw": embw,
        "svec": svec,
        "w1p": np.ascontiguousarray(w1p),
        "w2p": np.ascontiguousarray(w2p),
        "b2pack": b2pack,
    }


def kernel(**inputs):
    if "nc" not in _CACHE:
        _CACHE["nc"] = _build()
    nc = _CACHE["nc"]

    w = _prep_weights(inputs)
    bf = ml_dtypes.bfloat16
    xy = np.asarray(inputs["xy"], dtype=np.float32)
    t = np.asarray(inputs["t"], dtype=np.float32)

    coords = np.empty((6, B), bf)
    coords[0] = xy[:, 0].astype(bf)
    coords[1:3] = 1.0
    coords[3] = xy[:, 1].astype(bf)
    coords[4:6] = 1.0
    t_bf = t[:, 0].astype(bf)

    in_maps = []
    for c in range(NCORES):
        lo, hi = c * R, (c + 1) * R
        in_maps.append({
            "coords": np.ascontiguousarray(coords[:, lo:hi]),
            "tb": np.ascontiguousarray(
                np.broadcast_to(t_bf[lo:hi], (128, R))),
            **w,
        })

    res = run_bass_kernel_spmd(nc, in_maps, core_ids=list(range(NCORES)))
    _CACHE["last_res"] = res
    return np.concatenate([res.results[c]["out"] for c in range(NCORES)], axis=0)


# revision 7
# speedup vs baseline: 1.3594x; 1.2588x over previous
"""CourierEncoder fused kernel for 8 Trainium2 NeuronCores.

Data-parallel over the batch: each core processes B/8 = 32768 rows.
Per 512-row tile (matmuls bf16 -> fp32 PSUM), software-pipelined over a
3-stage skew (embeds for tile a=k, layer 1 for b=k-1, layer 2 for c=k-2):
  embeds:  x/y as K=3 outer products {w, b_hi, b_lo} (x) {coord, 1, 1} at
           row strips 0/32 (concurrent via tile_position auto-derive);
           cos folded as Sin(z+pi/2); biases folded into the matmul so both
           Sin activations fuse into ONE scalar-engine op (FD=1024).
           t-embed via host-broadcast tb[128,R] + DVE tensor_scalar
           (per-partition w_t/b_t) + stt LeakyReLU -- no PE, no PSUM.
  b2 bias: ones (x) b2hi/lo matmuls at row strip 96/64, concurrent with
           the x/y embed matmuls (distinct 32-row strips).
  layer 1: feature-major, 6 matmuls [128,128]@[128,512]; bias+LeakyReLU on
           DVE via tensor_scalar_add (fp32 per-partition b1) + SBUF stt
  layer 2: batch-major (lhsT = h1T slices); LeakyReLU on ACT via one
           Prelu op (FD=1024, single PSUM input)
"""

import math

import numpy as np
import ml_dtypes

import concourse.bass as bass
import concourse.tile as tile
import concourse.mybir as mybir
from concourse import bacc
from concourse.bass_utils import run_bass_kernel_spmd

B = 262144
NCORES = 8
R = B // NCORES          # rows per core
TILE = 512               # rows per tile
NT = R // TILE           # tiles per core
G = 4                    # tiles per input DMA group
PED = 256
NED = 128
CED = 256
Q = PED // 4             # 64
ALPHA = 0.01

F32 = mybir.dt.float32
BF16 = mybir.dt.bfloat16
AF = mybir.ActivationFunctionType
ALU = mybir.AluOpType

_CACHE = {}


def _build():
    nc = bacc.Bacc()
    coords = nc.dram_tensor("coords", [6, R], BF16, kind="ExternalInput")
    tb = nc.dram_tensor("tb", [128, R], BF16, kind="ExternalInput")
    embw = nc.dram_tensor("embw", [35, 128], BF16, kind="ExternalInput")
    svec = nc.dram_tensor("svec", [128, 4], F32, kind="ExternalInput")
    w1p = nc.dram_tensor("w1p", [128, 3, 2, 128], BF16, kind="ExternalInput")
    w2p = nc.dram_tensor("w2p", [128, 2, 256], BF16, kind="ExternalInput")
    b2pack = nc.dram_tensor("b2pack", [2, 640], BF16, kind="ExternalInput")
    out = nc.dram_tensor("out", [R, 256], F32, kind="ExternalOutput")

    with tile.TileContext(nc) as tc:
        with (
            tc.tile_pool(name="const", bufs=1) as const,
            tc.tile_pool(name="io", bufs=2) as io,
            tc.tile_pool(name="acts", bufs=3) as acts,
            tc.tile_pool(name="outp", bufs=4) as outp,
            tc.tile_pool(name="ps_emb", bufs=1, space="PSUM") as ps_emb,
            tc.tile_pool(name="ps_l1", bufs=1, space="PSUM") as ps_l1,
            tc.tile_pool(name="ps_l2", bufs=2, space="PSUM") as ps_l2,
        ):
            embw_sb = const.tile([35, 128], BF16)
            sv_sb = const.tile([128, 4], F32)   # wt, bt, b1c0, b1c1
            w1_sb = const.tile([128, 3, 2, 128], BF16)
            w2_sb = const.tile([128, 2, 256], BF16)
            b2_sb = const.tile([98, 640], BF16)
            nc.sync.dma_start(out=embw_sb, in_=embw[:, :])
            nc.sync.dma_start(out=sv_sb, in_=svec[:, :])
            nc.sync.dma_start(out=w1_sb, in_=w1p[:, :, :, :])
            nc.sync.dma_start(out=w2_sb, in_=w2p[:, :, :])
            nc.sync.dma_start(out=b2_sb[96:98, :], in_=b2pack[:, :])
            nc.sync.dma_start(out=b2_sb[64:66, :], in_=b2pack[:, :])

            xyin = [None] * (NT // G)
            tin = [None] * (NT // G)
            hxy = [None] * NT
            ht_ = [None] * NT
            h1T = [None] * NT
            l1ps = [None] * NT
            l2ps = [None] * NT

            for k in range(NT + 2):
                a = k          # stage A: embeds
                b = k - 1      # stage B: layer 1
                c = k - 2      # stage C: layer 2 + store

                if a < NT:
                    ga, ja = divmod(a, G)
                    if ja == 0:
                        lo, hi = ga * G * 512, (ga + 1) * G * 512
                        xyin[ga] = io.tile([35, G, 512], BF16, tag="xyin", name="xyin")
                        tin[ga] = io.tile([128, G, 512], BF16, tag="tin", name="tin")
                        for cc in range(2):
                            nc.sync.dma_start(
                                out=xyin[ga][32 * cc:32 * cc + 3, :, :],
                                in_=coords[3 * cc:3 * cc + 3, lo:hi].rearrange(
                                    "p (g n) -> p g n", n=512),
                            )
                        nc.sync.dma_start(
                            out=tin[ga],
                            in_=tb[:, lo:hi].rearrange("p (g n) -> p g n", n=512),
                        )

                    # strip matmuls: x-emb(0), y-emb(32), b2 bias(64, 96)
                    if c >= 0:
                        l2ps[c] = ps_l2.tile([128, 4, 256], F32, tag="l2", name="l2ps")
                        nc.tensor.matmul(
                            l2ps[c][:, 0:2, :],
                            b2_sb[96:98, 0:128], b2_sb[96:98, 128:640],
                            start=True, stop=False,
                            skip_group_check=True, tile_position=(96, 0),
                        )
                        nc.tensor.matmul(
                            l2ps[c][:, 2:4, :],
                            b2_sb[64:66, 0:128], b2_sb[64:66, 128:640],
                            start=True, stop=False,
                            skip_group_check=True, tile_position=(64, 0),
                        )
                    emb_ps = ps_emb.tile([128, 2, 512], F32)
                    for cc in range(2):
                        nc.tensor.matmul(
                            emb_ps[:, cc, :],
                            embw_sb[32 * cc:32 * cc + 3, :],
                            xyin[ga][32 * cc:32 * cc + 3, ja, :],
                            start=True, stop=True,
                        )
                    hxy[a] = acts.tile([128, 2, 512], BF16, tag="hxy", name="hxy")
                    nc.scalar.activation(out=hxy[a], in_=emb_ps, func=AF.Sin)
                    # t-embed fully on DVE from broadcast tb
                    zt = acts.tile([128, 512], BF16, tag="zt")
                    ht_[a] = acts.tile([128, 512], BF16, tag="ht", name="ht")
                    nc.vector.tensor_scalar(
                        out=zt, in0=tin[ga][:, ja, :],
                        scalar1=sv_sb[:, 0:1], scalar2=sv_sb[:, 1:2],
                        op0=ALU.mult, op1=ALU.add)
                    nc.vector.scalar_tensor_tensor(
                        out=ht_[a], in0=zt, scalar=ALPHA, in1=zt,
                        op0=ALU.mult, op1=ALU.max)
                elif c >= 0:
                    l2ps[c] = ps_l2.tile([128, 4, 256], F32, tag="l2", name="l2ps")
                    for h in range(2):
                        nc.tensor.matmul(
                            l2ps[c][:, 2 * h:2 * h + 2, :],
                            b2_sb[96:98, 0:128], b2_sb[96:98, 128:640],
                            start=True, stop=False,
                            skip_group_check=True, tile_position=(96, 0),
                        )

                # -- stage B: layer 1 (feature-major) -----------------------
                if 0 <= b < NT:
                    l1ps[b] = ps_l1.tile([128, 2, 512], F32, name="l1ps")
                    for mc in range(2):
                        for kc in range(2):
                            nc.tensor.matmul(
                                l1ps[b][:, mc, :],
                                w1_sb[:, kc, mc, :],
                                hxy[b][:, kc, :],
                                start=(kc == 0), stop=False,
                            )
                        nc.tensor.matmul(
                            l1ps[b][:, mc, :],
                            w1_sb[:, 2, mc, :],
                            ht_[b],
                            start=False, stop=True,
                        )
                    h1T[b] = acts.tile([128, 2, 512], BF16, tag="h1T", name="h1T")
                    # mc0 on ACT (Prelu w/ per-partition fp32 bias), mc1 on DVE
                    nc.scalar.activation(out=h1T[b][:, 0, :], in_=l1ps[b][:, 0, :],
                                         func=AF.Prelu, bias=sv_sb[:, 2:3],
                                         alpha=ALPHA)
                    tmp = acts.tile([128, 512], BF16, tag="tmp1")
                    nc.vector.tensor_scalar_add(
                        out=tmp, in0=l1ps[b][:, 1, :],
                        scalar1=sv_sb[:, 3:4])
                    nc.vector.scalar_tensor_tensor(
                        out=h1T[b][:, 1, :], in0=tmp, scalar=ALPHA,
                        in1=tmp, op0=ALU.mult, op1=ALU.max)

                # -- stage C: layer 2 (batch-major) + LeakyReLU + store -----
                if c >= 0:
                    for r in range(4):
                        for kc in range(2):
                            nc.tensor.matmul(
                                l2ps[c][:, r, :],
                                h1T[c][:, kc, r * 128:(r + 1) * 128],
                                w2_sb[:, kc, :],
                                start=False, stop=(kc == 1),
                                skip_group_check=True,
                            )
                    o_sb = outp.tile([128, 4, 256], F32)
                    nc.scalar.activation(out=o_sb, in_=l2ps[c],
                                         func=AF.Prelu, alpha=ALPHA)
                    base = c * TILE
                    nc.sync.dma_start(
                        out=out[base:base + TILE, :].rearrange(
                            "(r p) m -> p r m", p=128),
                        in_=o_sb,
                    )
                    hxy[c] = ht_[c] = h1T[c] = l1ps[c] = l2ps[c] = None
    nc.finalize()
    return nc


def _prep_weights(inputs):
    f = {k: np.asarray(v, dtype=np.float32) for k, v in inputs.items()}
    bf = ml_dtypes.bfloat16

    def hilo(v):
        hi = v.astype(bf).astype(np.float32)
        return hi.astype(bf), (v - hi).astype(bf)

    embw = np.zeros((35, 128), bf)
    embw[0] = np.concatenate([f["w_sx"].ravel(), f["w_cx"].ravel()])
    bx = np.concatenate([f["b_sx"], f["b_cx"] + math.pi / 2])
    embw[1], embw[2] = hilo(bx)
    embw[32] = np.concatenate([f["w_sy"].ravel(), f["w_cy"].ravel()])
    by = np.concatenate([f["b_sy"], f["b_cy"] + math.pi / 2])
    embw[33], embw[34] = hilo(by)

    svec = np.stack([
        np.repeat(f["w_t"].ravel(), 1),
        f["b_t"],
        f["b1"][0:128],
        f["b1"][128:256],
    ], axis=1)
    svec = np.ascontiguousarray(svec, dtype=np.float32)

    w1p = f["w1"].reshape(3, 128, 2, 128).transpose(1, 0, 2, 3).astype(bf)
    w2p = f["w2"].reshape(2, 128, 256).transpose(1, 0, 2).astype(bf)

    b2hi, b2lo = hilo(f["b2"])
    b2pack = np.zeros((2, 640), bf)
    b2pack[:, 0:128] = 1.0
    b2pack[0, 128:640] = np.concatenate([b2hi, b2hi])
    b2pack[1, 128:640] = np.concatenate([b2lo, b2lo])

    return {
        "embw": embw,
        "svec": svec,
        "w1p": np.ascontiguousarray(w1p),
        "w2p": np.ascontiguousarray(w2p),
        "b2pack": b2pack,
    }


def kernel(**inputs):
    if "nc" not in _CACHE:
        _CACHE["nc"] = _build()
    nc = _CACHE["nc"]

    w = _prep_weights(inputs)
    bf = ml_dtypes.bfloat16
    xy = np.asarray(inputs["xy"], dtype=np.float32)
    t = np.asarray(inputs["t"], dtype=np.float32)

    coords = np.empty((6, B), bf)
    coords[0] = xy[:, 0].astype(bf)
    coords[1:3] = 1.0
    coords[3] = xy[:, 1].astype(bf)
    coords[4:6] = 1.0
    t_bf = t[:, 0].astype(bf)

    in_maps = []
    for c in range(NCORES):
        lo, hi = c * R, (c + 1) * R
        in_maps.append({
            "coords": np.ascontiguousarray(coords[:, lo:hi]),
            "tb": np.ascontiguousarray(
                np.broadcast_to(t_bf[lo:hi], (128, R))),
            **w,
        })

    res = run_bass_kernel_spmd(nc, in_maps, core_ids=list(range(NCORES)))
    _CACHE["last_res"] = res
    return np.concatenate([res.results[c]["out"] for c in range(NCORES)], axis=0)
